# revision 4
# baseline (speedup 1.0000x reference)
"""Trainium2 Bass kernel for topic-aware LED decoder cross-attention.

Full inputs -> full output. Internally shards across 8 NeuronCores:
core c handles batch (c // 4) and heads (2*(c%4), 2*(c%4)+1).

Per-core kernel (all layouts transposed, [feature, token]):
  - cast f32->bf16 during DMA, transpose hs/kv/kvt via PE (is_transpose matmul)
  - q/k/v/kt/vt projections for the core's 2 heads
  - scores^T = k_h^T.T @ q_h^T per 128-row s-tile, exp on ACT (no max-sub:
    |scores| < ~3 by construction), softmax partition-sum via all-ones
    [128,128] stationary matmul (result is broadcast across partitions)
  - A = probs @ v, B = probs_t @ v, CT = probs_t @ vt (unnormalized, then
    multiplied by reciprocal row-sums)
  - partial gate logits L^T = sum_h Wg_h.T @ [A_h; CT_h; q_h]
  - D^T = sum_h (B_h @ Wo_h)^T  and  C_h^T = ((A_h - B_h) @ Wo_h)^T
Host side combines:  out_b^T = sum_cores D + sum_h sigmoid(L)_h * C_h, + bo.
This decomposition is exact (the gate is constant along the contracted dim)
and avoids any cross-core collectives.
"""

import numpy as np

B, T, S, E, TOPIC, H, DH = 2, 512, 4096, 1024, 512, 8, 128
HPC = 2           # heads per core
NCORES = 8
SC = 8            # s-chunks of 512
ST = S // 128     # 32 s-tiles of 128

_BUILT = {}


def build_bass():
    """Build (once) and return the compiled Bacc module."""
    if "nc" in _BUILT:
        return _BUILT["nc"]
    import warnings
    warnings.filterwarnings("ignore")
    import concourse.bass as bass
    import concourse.mybir as mybir
    import concourse.tile as tile
    from concourse import bacc
    from concourse.masks import make_identity

    f32 = mybir.dt.float32
    bf16 = mybir.dt.bfloat16
    Exp = mybir.ActivationFunctionType.Exp

    nc = bacc.Bacc("TRN2", target_bir_lowering=False, debug=False,
                   num_devices=NCORES)

    hs = nc.dram_tensor("hs", [T, E], f32, kind="ExternalInput")
    kv = nc.dram_tensor("kv", [S, E], f32, kind="ExternalInput")
    kvt = nc.dram_tensor("kvt", [S, TOPIC], f32, kind="ExternalInput")
    wq = nc.dram_tensor("wq", [E, 256], f32, kind="ExternalInput")
    wk = nc.dram_tensor("wk", [E, 256], f32, kind="ExternalInput")
    wv = nc.dram_tensor("wv", [E, 256], f32, kind="ExternalInput")
    wtk = nc.dram_tensor("wtk", [TOPIC, 256], f32, kind="ExternalInput")
    wtv = nc.dram_tensor("wtv", [TOPIC, 256], f32, kind="ExternalInput")
    wg = nc.dram_tensor("wg", [6 * 128, 8], f32, kind="ExternalInput")
    wo = nc.dram_tensor("wo", [256, E], f32, kind="ExternalInput")

    Dt = nc.dram_tensor("Dt", [E, T], f32, kind="ExternalOutput")
    C0t = nc.dram_tensor("C0t", [E, T], f32, kind="ExternalOutput")
    C1t = nc.dram_tensor("C1t", [E, T], f32, kind="ExternalOutput")
    Lp = nc.dram_tensor("Lp", [8, T], f32, kind="ExternalOutput")

    with tile.TileContext(nc) as tc:
        with (
            tc.tile_pool(name="const", bufs=1) as cst,
            tc.tile_pool(name="rot", bufs=2) as rot,
            tc.tile_pool(name="upool", bufs=8) as upool,
            tc.tile_pool(name="stage", bufs=3) as stg,
            tc.tile_pool(name="ps", bufs=8, space="PSUM") as psp,
        ):
            # ---- constants & weights ----
            ident = cst.tile([128, 128], bf16, tag="ident")
            make_identity(nc, ident[:])
            ones = cst.tile([128, 128], bf16, tag="ones")
            nc.vector.memset(ones[:], 1.0)

            wq_sb = cst.tile([128, 8, 256], bf16, tag="wq")
            nc.gpsimd.dma_start(out=wq_sb[:], in_=wq[:].rearrange("(a p) d -> p a d", p=128))
            wk_sb = cst.tile([128, 8, 256], bf16, tag="wk")
            nc.gpsimd.dma_start(out=wk_sb[:], in_=wk[:].rearrange("(a p) d -> p a d", p=128))
            wv_sb = cst.tile([128, 8, 256], bf16, tag="wv")
            nc.gpsimd.dma_start(out=wv_sb[:], in_=wv[:].rearrange("(a p) d -> p a d", p=128))
            wtk_sb = cst.tile([128, 4, 256], bf16, tag="wtk")
            nc.gpsimd.dma_start(out=wtk_sb[:], in_=wtk[:].rearrange("(a p) d -> p a d", p=128))
            wtv_sb = cst.tile([128, 4, 256], bf16, tag="wtv")
            nc.gpsimd.dma_start(out=wtv_sb[:], in_=wtv[:].rearrange("(a p) d -> p a d", p=128))
            wg_sb = cst.tile([128, 6, 8], bf16, tag="wg")
            nc.gpsimd.dma_start(out=wg_sb[:], in_=wg[:].rearrange("(a p) d -> p a d", p=128))
            wo_sb = cst.tile([128, 2, E], bf16, tag="wo")
            nc.gpsimd.dma_start(out=wo_sb[:], in_=wo[:].rearrange("(a p) d -> p a d", p=128))

            # ---- hs -> hsT (bf16) ----
            hs_nat = cst.tile([128, 4, E], bf16, tag="hs_nat")
            nc.gpsimd.dma_start(out=hs_nat[:], in_=hs[:].rearrange("(a p) e -> p a e", p=128))
            hsT = cst.tile([128, 8, T], bf16, tag="hsT")
            for a in range(4):
                for eb in range(8):
                    pt = psp.tile([128, 128], bf16, tag="ps")
                    nc.tensor.transpose(pt[:], hs_nat[:, a, eb * 128:(eb + 1) * 128], ident[:])
                    nc.vector.tensor_copy(out=hsT[:, eb, a * 128:(a + 1) * 128], in_=pt[:])

            # ---- qT (scaled via host-prescaled wq) ----
            qT = cst.tile([128, 2, T], bf16, tag="qT")
            for db in range(2):
                pq = psp.tile([128, 512], f32, tag="ps")
                for eb in range(8):
                    nc.tensor.matmul(pq[:], wq_sb[:, eb, db * 128:(db + 1) * 128],
                                     hsT[:, eb, :], start=(eb == 0), stop=(eb == 7))
                nc.vector.tensor_copy(out=qT[:, db, :], in_=pq[:])

            # ---- k/kt/v/vt projections, streaming over s-chunks ----
            kT = cst.tile([128, 2, S], bf16, tag="kT")
            ktT = cst.tile([128, 2, S], bf16, tag="ktT")
            v_sb = cst.tile([128, ST, 256], bf16, tag="v")
            vt_sb = cst.tile([128, ST, 256], bf16, tag="vt")
            for sc in range(SC):
                ssl = slice(sc * 512, (sc + 1) * 512)
                kv_nat = rot.tile([128, 4, E], bf16, tag="kv_nat")
                nc.gpsimd.dma_start(out=kv_nat[:], in_=kv[ssl, :].rearrange("(a p) e -> p a e", p=128))
                kvt_nat = rot.tile([128, 4, TOPIC], bf16, tag="kvt_nat")
                nc.gpsimd.dma_start(out=kvt_nat[:], in_=kvt[ssl, :].rearrange("(a p) e -> p a e", p=128))

                kvT = rot.tile([128, 8, 512], bf16, tag="kvT")
                for a in range(4):
                    for eb in range(8):
                        pt = psp.tile([128, 128], bf16, tag="ps")
                        nc.tensor.transpose(pt[:], kv_nat[:, a, eb * 128:(eb + 1) * 128], ident[:])
                        nc.vector.tensor_copy(out=kvT[:, eb, a * 128:(a + 1) * 128], in_=pt[:])
                kvtT = rot.tile([128, 4, 512], bf16, tag="kvtT")
                for a in range(4):
                    for eb in range(4):
                        pt = psp.tile([128, 128], bf16, tag="ps")
                        nc.tensor.transpose(pt[:], kvt_nat[:, a, eb * 128:(eb + 1) * 128], ident[:])
                        nc.vector.tensor_copy(out=kvtT[:, eb, a * 128:(a + 1) * 128], in_=pt[:])

                for db in range(2):
                    pk = psp.tile([128, 512], f32, tag="ps")
                    for eb in range(8):
                        nc.tensor.matmul(pk[:], wk_sb[:, eb, db * 128:(db + 1) * 128],
                                         kvT[:, eb, :], start=(eb == 0), stop=(eb == 7))
                    nc.vector.tensor_copy(out=kT[:, db, ssl], in_=pk[:])
                for db in range(2):
                    pk = psp.tile([128, 512], f32, tag="ps")
                    for eb in range(4):
                        nc.tensor.matmul(pk[:], wtk_sb[:, eb, db * 128:(db + 1) * 128],
                                         kvtT[:, eb, :], start=(eb == 0), stop=(eb == 3))
                    nc.vector.tensor_copy(out=ktT[:, db, ssl], in_=pk[:])
                for a in range(4):
                    pv = psp.tile([128, 512], f32, tag="ps")
                    for eb in range(8):
                        nc.tensor.matmul(pv[:, 0:256], kvT[:, eb, a * 128:(a + 1) * 128],
                                         wv_sb[:, eb, :], start=(eb == 0), stop=(eb == 7))
                    nc.vector.tensor_copy(out=v_sb[:, sc * 4 + a, :], in_=pv[:, 0:256])
                for a in range(4):
                    pv = psp.tile([128, 512], f32, tag="ps")
                    for eb in range(4):
                        nc.tensor.matmul(pv[:, 0:256], kvtT[:, eb, a * 128:(a + 1) * 128],
                                         wtv_sb[:, eb, :], start=(eb == 0), stop=(eb == 3))
                    nc.vector.tensor_copy(out=vt_sb[:, sc * 4 + a, :], in_=pv[:, 0:256])

            # ---- attention per head ----
            A_sb, B_sb, CT_sb, AmB_sb = {}, {}, {}, {}
            for h in range(HPC):
                pa = psp.tile([128, 512], f32, tag="ps")
                pb = psp.tile([128, 512], f32, tag="ps")
                pct = psp.tile([128, 512], f32, tag="ps")
                pr = psp.tile([128, 512], f32, tag="ps")
                prt = psp.tile([128, 512], f32, tag="ps")
                for st in range(ST):
                    first, last = (st == 0), (st == ST - 1)
                    pscr = psp.tile([128, 512], f32, tag="ps")
                    nc.tensor.matmul(pscr[:], kT[:, h, st * 128:(st + 1) * 128], qT[:, h, :],
                                     start=True, stop=True)
                    u = upool.tile([128, 512], bf16, tag="u")
                    nc.scalar.activation(out=u[:], in_=pscr[:], func=Exp)
                    pscr2 = psp.tile([128, 512], f32, tag="ps")
                    nc.tensor.matmul(pscr2[:], ktT[:, h, st * 128:(st + 1) * 128], qT[:, h, :],
                                     start=True, stop=True)
                    ut = upool.tile([128, 512], bf16, tag="ut")
                    nc.scalar.activation(out=ut[:], in_=pscr2[:], func=Exp)

                    nc.tensor.matmul(pr[:], ones[:], u[:], start=first, stop=last)
                    nc.tensor.matmul(pa[:], v_sb[:, st, h * 128:(h + 1) * 128], u[:],
                                     start=first, stop=last)
                    nc.tensor.matmul(prt[:], ones[:], ut[:], start=first, stop=last)
                    nc.tensor.matmul(pb[:], v_sb[:, st, h * 128:(h + 1) * 128], ut[:],
                                     start=first, stop=last)
                    nc.tensor.matmul(pct[:], vt_sb[:, st, h * 128:(h + 1) * 128], ut[:],
                                     start=first, stop=last)

                rinv = stg.tile([128, 512], f32, tag="rinv")
                nc.vector.reciprocal(out=rinv[:], in_=pr[:])
                rtinv = stg.tile([128, 512], f32, tag="rtinv")
                nc.vector.reciprocal(out=rtinv[:], in_=prt[:])

                A_sb[h] = cst.tile([128, 512], bf16, tag=f"A{h}", name=f"A{h}")
                nc.vector.tensor_mul(out=A_sb[h][:], in0=pa[:], in1=rinv[:])
                B_sb[h] = cst.tile([128, 512], bf16, tag=f"B{h}", name=f"B{h}")
                nc.vector.tensor_mul(out=B_sb[h][:], in0=pb[:], in1=rtinv[:])
                CT_sb[h] = cst.tile([128, 512], bf16, tag=f"CT{h}", name=f"CT{h}")
                nc.vector.tensor_mul(out=CT_sb[h][:], in0=pct[:], in1=rtinv[:])
                AmB_sb[h] = cst.tile([128, 512], bf16, tag=f"AmB{h}", name=f"AmB{h}")
                nc.vector.tensor_sub(out=AmB_sb[h][:], in0=A_sb[h][:], in1=B_sb[h][:])

            # ---- partial gate logits ----
            pl = psp.tile([128, 512], f32, tag="ps")
            gate_rhs = [A_sb[0], CT_sb[0], None, A_sb[1], CT_sb[1], None]
            for i in range(6):
                rhs = gate_rhs[i]
                rhs_ap = rhs[:] if rhs is not None else qT[:, i // 3, :]
                nc.tensor.matmul(pl[0:8, :], wg_sb[:, i, :], rhs_ap,
                                 start=(i == 0), stop=(i == 5))
            lp_sb = stg.tile([8, 512], f32, tag="lp")
            nc.vector.tensor_copy(out=lp_sb[:], in_=pl[0:8, :])
            nc.sync.dma_start(out=Lp[:], in_=lp_sb[:])

            # ---- output projections ----
            for eb in range(8):
                esl = slice(eb * 128, (eb + 1) * 128)
                po = psp.tile([128, 512], f32, tag="ps")
                nc.tensor.matmul(po[:], wo_sb[:, 0, esl], B_sb[0][:], start=True, stop=False)
                nc.tensor.matmul(po[:], wo_sb[:, 1, esl], B_sb[1][:], start=False, stop=True)
                so = stg.tile([128, 512], f32, tag="so")
                nc.vector.tensor_copy(out=so[:], in_=po[:])
                nc.sync.dma_start(out=Dt[esl, :], in_=so[:])
                for h, cdram in ((0, C0t), (1, C1t)):
                    pc = psp.tile([128, 512], f32, tag="ps")
                    nc.tensor.matmul(pc[:], wo_sb[:, h, esl], AmB_sb[h][:], start=True, stop=True)
                    sc2 = stg.tile([128, 512], f32, tag="so")
                    nc.vector.tensor_copy(out=sc2[:], in_=pc[:])
                    nc.sync.dma_start(out=cdram[esl, :], in_=sc2[:])

    nc.compile()
    _BUILT["nc"] = nc
    return nc


def _core_inputs(inputs, c):
    b, i = c // 4, c % 4
    h0 = 2 * i
    scale = np.float32(DH ** -0.5)
    f = np.ascontiguousarray
    dsl = slice(h0 * DH, (h0 + 2) * DH)
    return {
        "hs": f(inputs["hidden_states"][b]),
        "kv": f(inputs["key_value_states"][b]),
        "kvt": f(inputs["key_value_states_topical"][b]),
        "wq": f(inputs["Wq"][:, dsl] * scale),
        "wk": f(inputs["Wk"][:, dsl]),
        "wv": f(inputs["Wv"][:, dsl]),
        "wtk": f(inputs["Wtk"][:, dsl]),
        "wtv": f(inputs["Wtv"][:, dsl]),
        "wg": f(inputs["Wg"][h0 * 3 * DH:(h0 + 2) * 3 * DH, :]),
        "wo": f(inputs["Wo"][dsl, :]),
    }


def kernel(**inputs):
    from concourse.bass_utils import run_bass_kernel_spmd

    for name in ("bq", "bk", "bv", "btk", "btv"):
        assert not np.any(inputs[name]), f"nonzero bias {name} unsupported"

    nc = build_bass()
    in_maps = [_core_inputs(inputs, c) for c in range(NCORES)]
    res = run_bass_kernel_spmd(nc, in_maps, core_ids=list(range(NCORES)))

    bg = np.asarray(inputs["bg"], np.float32)
    bo = np.asarray(inputs["bo"], np.float32)
    out = np.empty((B, T, E), np.float32)
    for b in range(B):
        cores = range(b * 4, b * 4 + 4)
        L = sum(np.asarray(res.results[c]["Lp"], np.float32).T for c in cores) + bg
        gate = 1.0 / (1.0 + np.exp(-L))          # [T, H]
        outT = np.zeros((E, T), np.float32)
        for c in cores:
            h0 = 2 * (c % 4)
            r = res.results[c]
            outT += np.asarray(r["Dt"], np.float32)
            outT += gate[:, h0][None, :] * np.asarray(r["C0t"], np.float32)
            outT += gate[:, h0 + 1][None, :] * np.asarray(r["C1t"], np.float32)
        out[b] = outT.T + bo
    return out
